# revision 1
# baseline (speedup 1.0000x reference)
"""GQA causal attention (B=2, T=2048, C=2048, 32 Q heads, 8 KV heads) on 8
Trainium2 NeuronCores.

Sharding: tensor-parallel over KV-head groups. Core i owns KV head i and its
4 query heads: it computes q/k/v projections for its heads (256/64/64 output
channels), flash-style causal attention in scores-transposed layout, then the
cores AllGather the (normalized) attention output in head-major transposed
layout [C, B*T] and each core computes a 256-column slice of the final
projection. Host concatenates the column slices.

Layout notes:
  - x is fed pre-transposed as xT [C, B*T] so every projection matmul
    contracts C on the partition dimension without on-device transposes.
  - Scores are computed transposed (sT [k, q]) so softmax summation is a
    ones-column matmul and no per-block transposes are needed; V is needed in
    natural [t, d] layout and is produced by PE-transposing the vT projection.
  - All matmuls run in float32r (full-rate fp32, ~5e-6 rel rounding).
"""

import sys

sys.path.insert(0, "/opt/trn_rl_repo")

import numpy as np
import ml_dtypes

import concourse.bass as bass
import concourse.mybir as mybir
import concourse.tile as tile

P = 128
B, T, C = 2, 2048, 2048
BT = B * T            # 4096
NH, NKV = 32, 8
HD = C // NH          # 64
G = NH // NKV         # 4 q heads per kv head / per core
CQ = G * HD           # 256 q/out channels per core
KC = C // P           # 16 contraction chunks
TQ = 512              # t-chunk
NCORES = 8

f32 = mybir.dt.float32
f32r = mybir.dt.float32r
bf16 = mybir.dt.bfloat16
EXP = mybir.ActivationFunctionType.Exp
SCALE = float(HD) ** -0.5


def split_multi_waits(nc):
    """Walrus codegen allows only one sync-wait per engine instruction; move
    extras onto standalone same-engine EventSemaphore waits placed before."""
    for fn in nc.m.functions:
        for bb in fn.blocks:
            out = []
            for inst in bb.instructions:
                si = inst.sync_info
                if si is not None and si.on_wait and len(si.on_wait) > 1:
                    waits = list(si.on_wait)
                    for j, w in enumerate(waits[:-1]):
                        nop = mybir.InstEventSemaphore(
                            name=f"{inst.name}-ws{j}", ins=[], outs=[],
                            engine=inst.engine)
                        nop.sync_info = mybir.SyncInfo(on_wait=[w], on_update=[])
                        out.append(nop)
                    inst.sync_info = mybir.SyncInfo(
                        on_wait=[waits[-1]], on_update=list(si.on_update))
                out.append(inst)
            try:
                bb.instructions[:] = out
            except TypeError:
                bb.instructions.clear()
                bb.instructions.extend(out)


def build():
    nc = bass.Bass(num_devices=NCORES)

    xt_d = nc.dram_tensor("xt", [C, BT], f32r, kind="ExternalInput")
    wq_d = nc.dram_tensor("wq", [C, CQ], f32r, kind="ExternalInput")
    wkv_d = nc.dram_tensor("wkv", [C, P], f32r, kind="ExternalInput")
    wp_d = nc.dram_tensor("wp", [C, CQ], bf16, kind="ExternalInput")
    bpb_d = nc.dram_tensor("bpb", [P, CQ], f32, kind="ExternalInput")
    mask_d = nc.dram_tensor("masks", [P, 4 * TQ], f32r, kind="ExternalInput")
    idn_d = nc.dram_tensor("ident", [P, P], f32, kind="ExternalInput")
    ones_d = nc.dram_tensor("ones", [1, HD], f32r, kind="ExternalInput")
    vpad_d = nc.dram_tensor("vpad", [P, 2], f32r, kind="ExternalInput")
    out_d = nc.dram_tensor("out", [BT, CQ], f32, kind="ExternalOutput")

    with tile.TileContext(nc) as tc:
        with tc.tile_pool(name="res", bufs=1) as res, \
             tc.tile_pool(name="dram", bufs=1, space="DRAM") as dp:
            ones_sb = res.tile([1, HD], f32r)
            nc.sync.dma_start(ones_sb[:], ones_d[:, :])

            # long-lived activations; one qT tile per head so every matmul
            # operand sits at base partition 0
            qTh = [res.tile([HD, BT], f32r, name=f"qt{h}") for h in range(G)]
            kT = res.tile([HD, BT], f32r)
            va = res.tile([P, BT // P, HD + 2], f32r)  # v natural + ones col
            for kb in range(BT // P):
                nc.sync.dma_start(va[:, kb, HD:HD + 2], vpad_d[:, :])
            yU = res.tile([HD + 1, 32 * TQ], f32)  # unnormalized y (+l row)
            lA = res.tile([32, TQ], f32)
            rA = res.tile([32, TQ], f32)
            yt_loc = dp.tile([CQ, BT], bf16)
            yt_ag = dp.tile([NCORES * CQ, BT], bf16, addr_space="Shared")

            # ---- Phase 1: q/k/v projections (contract C on partitions) ----
            with tc.tile_pool(name="xp", bufs=5) as xp, \
                 tc.tile_pool(name="w1", bufs=1) as w1, \
                 tc.tile_pool(name="pps", bufs=2, space="PSUM") as pps, \
                 tc.tile_pool(name="tps", bufs=2, space="PSUM") as tps:
                wq_sb = w1.tile([P, KC, CQ], f32r)
                nc.sync.dma_start(wq_sb[:], wq_d.rearrange("(o p) n -> p o n", p=P))
                wkv_sb = w1.tile([P, KC, P], f32r)
                nc.sync.dma_start(wkv_sb[:], wkv_d.rearrange("(o p) n -> p o n", p=P))
                idn_sb = w1.tile([P, P], f32)
                nc.sync.dma_start(idn_sb[:], idn_d[:, :])
                for tb in range(BT // TQ):
                    q0_ps = pps.tile([P, TQ], f32, tag="q0")
                    q1_ps = pps.tile([P, TQ], f32, tag="q1")
                    kv_ps = pps.tile([P, TQ], f32, tag="kv")
                    for c in range(KC):
                        xt_t = xp.tile([P, TQ], f32r, tag="xt")
                        nc.sync.dma_start(
                            xt_t[:], xt_d[c * P:(c + 1) * P, tb * TQ:(tb + 1) * TQ])
                        nc.tensor.matmul(q0_ps[:], wq_sb[:, c, 0:P], xt_t[:],
                                         start=(c == 0), stop=(c == KC - 1))
                        nc.tensor.matmul(q1_ps[:], wq_sb[:, c, P:CQ], xt_t[:],
                                         start=(c == 0), stop=(c == KC - 1))
                        nc.tensor.matmul(kv_ps[:], wkv_sb[:, c, :], xt_t[:],
                                         start=(c == 0), stop=(c == KC - 1))
                    sl = slice(tb * TQ, (tb + 1) * TQ)
                    nc.vector.tensor_copy(qTh[0][:, sl], q0_ps[0:HD, :])
                    nc.vector.tensor_copy(qTh[1][:, sl], q0_ps[HD:P, :])
                    nc.vector.tensor_copy(qTh[2][:, sl], q1_ps[0:HD, :])
                    nc.vector.tensor_copy(qTh[3][:, sl], q1_ps[HD:P, :])
                    nc.vector.tensor_copy(kT[:, sl], kv_ps[0:HD, :])
                    vs_t = xp.tile([HD, TQ], f32, tag="vs")
                    nc.vector.tensor_copy(vs_t[:], kv_ps[HD:P, :])
                    # V natural layout via PE transpose of vT blocks
                    for k4 in range(TQ // P):
                        kb = tb * (TQ // P) + k4
                        vt_ps = tps.tile([P, HD], f32, tag="vt")
                        nc.tensor.transpose(vt_ps[:], vs_t[:, k4 * P:(k4 + 1) * P],
                                            idn_sb[0:HD, 0:HD])
                        nc.vector.tensor_copy(va[:, kb, 0:HD], vt_ps[:])

            # ---- Phase 2: causal attention, scores-transposed layout ----
            with tc.tile_pool(name="aps", bufs=3, space="PSUM") as aps, \
                 tc.tile_pool(name="yps", bufs=2, space="PSUM") as yps, \
                 tc.tile_pool(name="ep", bufs=8) as ep:
                mask_sb = ep.tile([P, 4 * TQ], f32r, tag="mk", bufs=1)
                nc.sync.dma_start(mask_sb[:], mask_d[:, :])
                for b in range(B):
                    for qh in range(G):
                        for qc in range(T // TQ):
                            idx = (b * G + qh) * 4 + qc
                            nkb = 4 * qc + 4
                            y_ps = yps.tile([HD + 2, TQ], f32, tag="y")
                            qap = qTh[qh][:, b * T + qc * TQ:
                                          b * T + (qc + 1) * TQ]
                            for kbp in range(nkb // 2):
                                s_ps = aps.tile([P, 2 * TQ], f32, tag="s")
                                for h in range(2):
                                    kb = kbp * 2 + h
                                    nc.tensor.matmul(
                                        s_ps[:, h * TQ:(h + 1) * TQ],
                                        kT[:, b * T + kb * P: b * T + (kb + 1) * P],
                                        qap, start=True, stop=True)
                                ex = ep.tile([P, 2 * TQ], f32r, tag="ex")
                                nc.scalar.activation(ex[:], s_ps[:], EXP,
                                                     scale=SCALE)
                                for h in range(2):
                                    kb = kbp * 2 + h
                                    j = kb - 4 * qc
                                    exh = ex[:, h * TQ:(h + 1) * TQ]
                                    if j >= 0:
                                        nc.vector.tensor_mul(
                                            exh, exh,
                                            mask_sb[:, j * TQ:(j + 1) * TQ])
                                    nc.tensor.matmul(
                                        y_ps[:], va[:, b * (T // P) + kb, :], exh,
                                        start=(kb == 0), stop=(kb == nkb - 1))
                            sl = slice(idx * TQ, (idx + 1) * TQ)
                            nc.vector.tensor_copy(yU[:, sl], y_ps[0:HD + 1, :])
                            nc.sync.dma_start(lA[idx:idx + 1, :], yU[HD:HD + 1, sl])

            # ---- Phase 3+4 pools: final-projection weights load early so
            # the DMA overlaps normalize + AllGather ----
            with tc.tile_pool(name="fp", bufs=4) as fp, \
                 tc.tile_pool(name="np_", bufs=4) as npo:
                bps_cm = tc.tile_pool(name="bps", bufs=2, space="PSUM")
                bps = bps_cm.__enter__()
                wp_sb = fp.tile([P, KC, CQ], bf16, tag="wp", bufs=1)
                nc.sync.dma_start(wp_sb[:], wp_d.rearrange("(o p) n -> p o n", p=P))
                bpb_sb = fp.tile([P, CQ], f32, tag="bp", bufs=1)
                nc.sync.dma_start(bpb_sb[:], bpb_d[:, :])
                nc.vector.reciprocal(rA[:], lA[:])
                for idx in range(32):
                    b, qh, qc = idx // 16, (idx // 4) % 4, idx % 4
                    rrow = npo.tile([1, TQ], f32r, tag="rr")
                    nc.sync.dma_start(rrow[:], rA[idx:idx + 1, :].bitcast(f32r))
                    bc_ps = bps.tile([HD, TQ], f32, tag="bc")
                    nc.tensor.matmul(bc_ps[:], ones_sb[:], rrow[:],
                                     start=True, stop=True)
                    yn = npo.tile([HD, TQ], bf16, tag="yn")
                    nc.vector.tensor_mul(yn[:], yU[0:HD, idx * TQ:(idx + 1) * TQ],
                                         bc_ps[:])
                    nc.sync.dma_start(
                        yt_loc[qh * HD:(qh + 1) * HD,
                               b * T + qc * TQ: b * T + (qc + 1) * TQ], yn[:])
                nc.gpsimd.collective_compute(
                    "AllGather", mybir.AluOpType.bypass,
                    replica_groups=[list(range(NCORES))],
                    ins=[yt_loc[:].opt()], outs=[yt_ag[:].opt()])

                bps_cm.__exit__(None, None, None)
                # ---- Phase 4: output projection (column slice) + bias ----
                with tc.tile_pool(name="fps", bufs=2, space="PSUM") as fps:
                  for tbo in range(BT // TQ):
                    o_ps = [fps.tile([P, CQ], f32, tag=f"o{i}", name=f"o{i}") for i in range(4)]
                    for c in range(KC):
                        yt_t = fp.tile([P, TQ], bf16, tag="yt")
                        nc.sync.dma_start(
                            yt_t[:], yt_ag[c * P:(c + 1) * P,
                                           tbo * TQ:(tbo + 1) * TQ])
                        for ti in range(4):
                            nc.tensor.matmul(
                                o_ps[ti][:], yt_t[:, ti * P:(ti + 1) * P],
                                wp_sb[:, c, :],
                                start=(c == 0), stop=(c == KC - 1))
                    for ti in range(4):
                        o_sb = fp.tile([P, CQ], f32, tag="ob")
                        nc.vector.tensor_add(o_sb[:], o_ps[ti][:], bpb_sb[:])
                        nc.sync.dma_start(
                            out_d[(tbo * 4 + ti) * P:(tbo * 4 + ti + 1) * P, :],
                            o_sb[:])

    split_multi_waits(nc)
    return nc


_NC_CACHE = None


def _get_nc():
    global _NC_CACHE
    if _NC_CACHE is None:
        _NC_CACHE = build()
    return _NC_CACHE


def make_in_maps(x, wq, wk, wv, wp, bp):
    x = np.asarray(x, dtype=np.float32)
    xt = np.ascontiguousarray(x.reshape(BT, C).T)
    masks = np.zeros((P, 4 * TQ), dtype=np.float32)
    for j in range(4):
        kk = np.arange(P)[:, None]
        qq = np.arange(TQ)[None, :]
        masks[:, j * TQ:(j + 1) * TQ] = (j * P + kk <= qq).astype(np.float32)
    ident = np.eye(P, dtype=np.float32)
    ones = np.ones((1, HD), dtype=np.float32)
    vpad = np.zeros((P, 2), dtype=np.float32)
    vpad[:, 0] = 1.0
    in_maps = []
    for i in range(NCORES):
        cs = slice(i * CQ, (i + 1) * CQ)
        ks = slice(i * HD, (i + 1) * HD)
        wkv = np.concatenate(
            [np.asarray(wk)[:, ks], np.asarray(wv)[:, ks]], axis=1)
        in_maps.append({
            "xt": xt,
            "wq": np.ascontiguousarray(np.asarray(wq, np.float32)[:, cs]),
            "wkv": np.ascontiguousarray(wkv.astype(np.float32)),
            "wp": np.ascontiguousarray(np.asarray(wp, np.float32)[:, cs]).astype(ml_dtypes.bfloat16),
            "bpb": np.tile(np.asarray(bp, np.float32)[None, cs], (P, 1)),
            "masks": masks,
            "ident": ident,
            "ones": ones,
            "vpad": vpad,
        })
    return in_maps


def kernel(x, wq, wk, wv, wp, bp, _trace=False):
    from concourse.bass_utils import run_bass_kernel_spmd
    nc = _get_nc()
    in_maps = make_in_maps(x, wq, wk, wv, wp, bp)
    res = run_bass_kernel_spmd(nc, in_maps, list(range(NCORES)), trace=_trace)
    out = np.concatenate([res.results[i]["out"] for i in range(NCORES)], axis=1)
    out = out.reshape(B, T, C).astype(np.float32)
    if _trace:
        return out, res
    return out



# revision 23
# speedup vs baseline: 4.1783x; 4.1783x over previous
"""GQA causal attention (B=2, T=2048, C=2048, 32 Q heads, 8 KV heads) on 8
Trainium2 NeuronCores.

Sharding: tensor-parallel attention over KV-head groups (core i owns KV head
i and its 4 query heads), then TOKEN-parallel output projection: instead of
AllGather-ing the full [C, BT] attention output (16MB collective), each core
AllToAll-exchanges normalized per-head outputs so core i ends up with
yT[all 2048 channels, its 512 tokens] (2MB collective, 8x less traffic) and
computes out[512 tokens, 2048 channels] with the full wp.

The A2A is staged per q-head (4 stages x 0.5MB) and overlaps attention
compute of the following head; the final projection starts on the first 3
stages' channels while the last A2A is still in flight.

Layouts/dtypes:
  - x fed pre-transposed as xt [C, BT] bf16; all matmul operands bf16
    (fp32 PSUM accumulate), so PE streams at full rate and DMA bytes halve.
  - scores computed transposed (s[k, q]) so softmax sum is a ones-column in
    the V matmul; V natural layout produced by PE-transposing vT blocks.
  - causal trimming: score/exp/yacc matmuls only cover the un-masked
    [off, TQ) column range of diagonal blocks.
  - wp is fed row-permuted (head-major) so each A2A stage's channels are
    contiguous contraction chunks.
"""

import sys

sys.path.insert(0, "/opt/trn_rl_repo")

import numpy as np
import ml_dtypes

import concourse.bass as bass
import concourse.mybir as mybir
import concourse.tile as tile

P = 128
B, T, C = 2, 2048, 2048
BT = B * T            # 4096
NH, NKV = 32, 8
HD = C // NH          # 64
G = NH // NKV         # 4 q heads per kv head / per core
CQ = G * HD           # 256 q channels per core
KC = C // P           # 16 contraction chunks
TQ = 512              # token-chunk
NCH = BT // TQ        # 8 token chunks == NCORES
NCORES = 8
NTOK = BT // NCORES   # 512 tokens per core for the output projection

f32 = mybir.dt.float32
f32r = mybir.dt.float32r
bf16 = mybir.dt.bfloat16
EXP = mybir.ActivationFunctionType.Exp
SCALE = float(HD) ** -0.5


def split_multi_waits(nc):
    """Walrus codegen allows only one sync-wait per engine instruction; move
    extras onto standalone same-engine EventSemaphore waits placed before."""
    for fn in nc.m.functions:
        for bb in fn.blocks:
            out = []
            for inst in bb.instructions:
                si = inst.sync_info
                if si is not None and si.on_wait and len(si.on_wait) > 1:
                    waits = list(si.on_wait)
                    for j, w in enumerate(waits[:-1]):
                        nop = mybir.InstEventSemaphore(
                            name=f"{inst.name}-ws{j}", ins=[], outs=[],
                            engine=inst.engine)
                        nop.sync_info = mybir.SyncInfo(on_wait=[w], on_update=[])
                        out.append(nop)
                    inst.sync_info = mybir.SyncInfo(
                        on_wait=[waits[-1]], on_update=list(si.on_update))
                out.append(inst)
            try:
                bb.instructions[:] = out
            except TypeError:
                bb.instructions.clear()
                bb.instructions.extend(out)


def build(n_rep=1):
    nc = bass.Bass(num_devices=NCORES)

    xt_d = nc.dram_tensor("xt", [C, BT], bf16, kind="ExternalInput")
    wq_d = nc.dram_tensor("wq", [C, CQ], bf16, kind="ExternalInput")
    wkv_d = nc.dram_tensor("wkv", [C, P], bf16, kind="ExternalInput")
    wpp_d = nc.dram_tensor("wpp", [C, C], bf16, kind="ExternalInput")
    bpb_d = nc.dram_tensor("bpb", [P, C], f32, kind="ExternalInput")
    mask_d = nc.dram_tensor("masks", [P, G * TQ], bf16, kind="ExternalInput")
    idn_d = nc.dram_tensor("ident", [P, P], f32, kind="ExternalInput")
    ones_d = nc.dram_tensor("ones", [1, HD], f32r, kind="ExternalInput")
    vpad_d = nc.dram_tensor("vpad", [P, (BT // P) * 2], bf16,
                            kind="ExternalInput")
    out_d = nc.dram_tensor("out", [NTOK, C], f32, kind="ExternalOutput")

    with tile.TileContext(nc) as tc:
      with tc.tile_pool(name="dram", bufs=1, space="DRAM") as dp:
        for rep in range(n_rep):
          with tc.tile_pool(name="act", bufs=1) as act:
            # phase-2/3 constants: tiles here, DMAs interleaved into phase 1
            # so the first xt/wq transfers aren't stuck behind them
            ones_sb = act.tile([1, HD], f32r)
            idn_sb = act.tile([P, P], f32)
            mask_sb = act.tile([P, G, TQ], bf16)
            bpb_sb = act.tile([P, C], f32)
            wp_sb = act.tile([P, KC, C], bf16)
            # long-lived activations; one qT tile per head so every matmul
            # operand sits at base partition 0
            qTh = [act.tile([HD, BT], bf16, name=f"qt{rep}_{h}")
                   for h in range(G)]
            kT = act.tile([HD, BT], bf16)
            va = act.tile([P, BT // P, HD + 2], bf16)  # v natural + ones col
            nc.sync.dma_start(
                va[:, :, HD:HD + 2],
                vpad_d.rearrange("p (k c) -> p k c", c=2))
            yt_sb = act.tile([P, KC, TQ], bf16)  # A2A result (proj lhsT)
            yt_loc = dp.tile([G, NCORES, HD, TQ], bf16)
            yt_a2a = dp.tile([G, NCORES * HD // P, P, TQ], bf16)

            # ---- Phase 1: q/k/v projections (contract C on partitions) ----
            with tc.tile_pool(name="xp", bufs=4) as xp, \
                 tc.tile_pool(name="w1", bufs=1) as w1, \
                 tc.tile_pool(name="pps", bufs=2, space="PSUM") as pps, \
                 tc.tile_pool(name="tps", bufs=2, space="PSUM") as tps:
                wq_sb = w1.tile([P, KC, CQ], bf16)
                nc.sync.dma_start(wq_sb[:], wq_d.rearrange("(o p) n -> p o n", p=P))
                wkv_sb = w1.tile([P, KC, P], bf16)
                nc.sync.dma_start(wkv_sb[:], wkv_d.rearrange("(o p) n -> p o n", p=P))
                # idn is consumed by tb=0's transposes: must be written first
                nc.sync.dma_start(idn_sb[:], idn_d[:, :])
                for tb in range(BT // TQ):
                    if tb == 1:
                        nc.sync.dma_start(ones_sb[:], ones_d[:, :])
                        nc.sync.dma_start(
                            mask_sb[:],
                            mask_d.rearrange("p (g t) -> p g t", g=G))
                        nc.sync.dma_start(bpb_sb[:], bpb_d[:, :])
                    elif tb >= 2:
                        # two 0.5MB wp chunks per tb: c0-11 by tb=7, rest after
                        for w2 in range(2):
                            cw = (tb - 2) * 2 + w2
                            nc.sync.dma_start(
                                wp_sb[:, cw, :], wpp_d[cw * P:(cw + 1) * P, :])
                    q0_ps = pps.tile([P, TQ], f32, tag="q0")
                    q1_ps = pps.tile([P, TQ], f32, tag="q1")
                    kv_ps = pps.tile([P, TQ], f32, tag="kv")
                    for cg in range(KC // 4):
                        xt_t = xp.tile([P, 4, TQ], bf16, tag="xt")
                        nc.sync.dma_start(
                            xt_t[:],
                            xt_d[cg * 4 * P:(cg + 1) * 4 * P,
                                 tb * TQ:(tb + 1) * TQ].rearrange(
                                "(o p) n -> p o n", p=P))
                        for cc in range(4):
                            c = cg * 4 + cc
                            nc.tensor.matmul(q0_ps[:], wq_sb[:, c, 0:P],
                                             xt_t[:, cc, :],
                                             start=(c == 0), stop=(c == KC - 1))
                            nc.tensor.matmul(q1_ps[:], wq_sb[:, c, P:CQ],
                                             xt_t[:, cc, :],
                                             start=(c == 0), stop=(c == KC - 1))
                            nc.tensor.matmul(kv_ps[:], wkv_sb[:, c, :],
                                             xt_t[:, cc, :],
                                             start=(c == 0), stop=(c == KC - 1))
                    sl = slice(tb * TQ, (tb + 1) * TQ)
                    nc.vector.tensor_copy(qTh[0][:, sl], q0_ps[0:HD, :])
                    nc.vector.tensor_copy(qTh[1][:, sl], q0_ps[HD:P, :])
                    nc.vector.tensor_copy(qTh[2][:, sl], q1_ps[0:HD, :])
                    nc.vector.tensor_copy(qTh[3][:, sl], q1_ps[HD:P, :])
                    nc.vector.tensor_copy(kT[:, sl], kv_ps[0:HD, :])
                    vs_t = xp.tile([HD, TQ], f32, tag="vs")
                    nc.vector.tensor_copy(vs_t[:], kv_ps[HD:P, :])
                    # V natural layout via PE transpose of vT blocks
                    for k4 in range(TQ // P):
                        kb = tb * (TQ // P) + k4
                        vt_ps = tps.tile([P, HD], f32, tag="vt")
                        nc.tensor.transpose(vt_ps[:], vs_t[:, k4 * P:(k4 + 1) * P],
                                            idn_sb[0:HD, 0:HD])
                        nc.vector.tensor_copy(va[:, kb, 0:HD], vt_ps[:])
                for cw in range(12, KC):
                    nc.sync.dma_start(
                        wp_sb[:, cw, :], wpp_d[cw * P:(cw + 1) * P, :])

            # ---- Phase 2: causal attention (scores-transposed, trimmed),
            # head-staged normalize + AllToAll overlapped with next head ----
            with tc.tile_pool(name="aps", bufs=2, space="PSUM") as aps, \
                 tc.tile_pool(name="yps", bufs=2, space="PSUM") as yps, \
                 tc.tile_pool(name="bps", bufs=2, space="PSUM") as bps, \
                 tc.tile_pool(name="ep", bufs=6) as ep, \
                 tc.tile_pool(name="np_", bufs=4) as npo:
                for qh in range(G):
                    for ci in range(NCH):
                        b, qc = ci // (T // TQ), ci % (T // TQ)
                        nkb = 4 * qc + 4
                        y_ps = yps.tile([HD + 2, TQ], f32, tag="y")
                        qap = qTh[qh][:, b * T + qc * TQ: b * T + (qc + 1) * TQ]
                        for kbp in range(nkb // 2):
                            j0 = 2 * kbp - 4 * qc
                            offp = P * max(0, j0)
                            s_ps = aps.tile([P, 2, TQ], f32, tag="s")
                            for h in range(2):
                                kb = 2 * kbp + h
                                off = P * max(0, kb - 4 * qc)
                                nc.tensor.matmul(
                                    s_ps[:, h, off:],
                                    kT[:, b * T + kb * P: b * T + (kb + 1) * P],
                                    qap[:, off:], start=True, stop=True)
                            ex = ep.tile([P, 2, TQ], bf16, tag="ex")
                            nc.scalar.activation(ex[:, :, offp:],
                                                 s_ps[:, :, offp:], EXP,
                                                 scale=SCALE)
                            for h in range(2):
                                kb = 2 * kbp + h
                                j = kb - 4 * qc
                                off = P * max(0, j)
                                if j >= 0:
                                    nc.vector.tensor_mul(
                                        ex[:, h, off:off + P],
                                        ex[:, h, off:off + P],
                                        mask_sb[:, j, off:off + P])
                                nc.tensor.matmul(
                                    y_ps[:, off:],
                                    va[:, b * (T // P) + kb, :],
                                    ex[:, h, off:],
                                    start=(kb == 0), stop=(kb == nkb - 1),
                                    skip_group_check=True)
                        # normalize this chunk right away: r = 1/l, broadcast
                        # down the 64 dims via ones-matmul, scale, ship out
                        rrow = npo.tile([1, TQ], f32r, tag="rr", bufs=2)
                        with nc.allow_low_precision(
                                reason="1/l as f32r matmul operand (~5e-6)"):
                            nc.vector.reciprocal(rrow[:], y_ps[HD:HD + 1, :])
                        ystage = npo.tile([HD, TQ], bf16, tag="ys", bufs=2)
                        nc.vector.tensor_copy(ystage[:], y_ps[0:HD, :])
                        bc_ps = bps.tile([HD, TQ], f32, tag="bc")
                        nc.tensor.matmul(bc_ps[:], ones_sb[:], rrow[:],
                                         start=True, stop=True)
                        yn = npo.tile([HD, TQ], bf16, tag="yn")
                        nc.vector.tensor_mul(yn[:], ystage[:], bc_ps[:])
                        nc.sync.dma_start(yt_loc[qh, ci, :, :], yn[:])
                    nc.gpsimd.collective_compute(
                        "AllToAll", mybir.AluOpType.bypass,
                        replica_groups=[list(range(NCORES))],
                        ins=[yt_loc[qh].opt()], outs=[yt_a2a[qh].opt()])
                    # land stage channels for the projection
                    nc.sync.dma_start(
                        yt_sb[:, qh * G:(qh + 1) * G, :],
                        yt_a2a[qh].rearrange("c p t -> p c t"))

            # ---- Phase 3: token-sharded output projection + bias ----
            # Column-groups of 512 out-channels x all 4 token blocks (4 PSUM
            # banks each, 2 groups in flight). All groups pre-accumulate the
            # A2A stage-0..2 channels (c0-11) while the last A2A is in
            # flight; groups 0/1 park their partials in SBUF and preload
            # them back into PSUM for the c12-15 finish.
            NG = C // TQ
            with tc.tile_pool(name="fps", bufs=2, space="PSUM") as fps, \
                 tc.tile_pool(name="fp", bufs=2) as fp:
                keep = {}
                dumps = {}
                CPRE = 12  # chunks from A2A stages 0-2

                def proj_chunks(ops, g, c0, c1, start):
                    osl = slice(g * TQ, (g + 1) * TQ)
                    for c in range(c0, c1):
                        for i in range(4):
                            nc.tensor.matmul(
                                ops[i][:], yt_sb[:, c, i * P:(i + 1) * P],
                                wp_sb[:, c, osl],
                                start=start and (c == c0), stop=(c == c1 - 1),
                                skip_group_check=True)

                for g in range(NG):
                    ops = [fps.tile([P, TQ], f32, tag=f"t{i}",
                                    name=f"o{rep}_{g}_{i}") for i in range(4)]
                    proj_chunks(ops, g, 0, CPRE, start=True)
                    if g < 2:
                        dsb = fp.tile([P, 4, TQ], f32, tag=f"d{g}", bufs=1,
                                      name=f"d{rep}_{g}")
                        for i in range(4):
                            nc.vector.tensor_copy(dsb[:, i, :], ops[i][:])
                        dumps[g] = dsb
                    else:
                        keep[g] = ops
                for g in (2, 3, 0, 1):
                    osl = slice(g * TQ, (g + 1) * TQ)
                    if g < 2:
                        ops = [fps.tile([P, TQ], f32, tag=f"t{i}",
                                        name=f"p{rep}_{g}_{i}")
                               for i in range(4)]
                        for i in range(4):
                            nc.vector.tensor_copy(ops[i][:], dumps[g][:, i, :])
                    else:
                        ops = keep[g]
                    proj_chunks(ops, g, CPRE, KC, start=False)
                    for i in range(4):
                        o_sb = fp.tile([P, TQ], f32, tag="ob", bufs=4)
                        nc.vector.tensor_add(o_sb[:], ops[i][:],
                                             bpb_sb[:, osl])
                        nc.sync.dma_start(
                            out_d[i * P:(i + 1) * P, osl], o_sb[:])

    split_multi_waits(nc)
    return nc


_NC_CACHE = {}


def _get_nc(n_rep=1):
    if n_rep not in _NC_CACHE:
        _NC_CACHE[n_rep] = build(n_rep)
    return _NC_CACHE[n_rep]


def make_in_maps(x, wq, wk, wv, wp, bp):
    x = np.asarray(x, dtype=np.float32)
    xt = np.ascontiguousarray(x.reshape(BT, C).T).astype(ml_dtypes.bfloat16)
    masks = np.zeros((P, G * TQ), dtype=np.float32)
    for j in range(G):
        kk = np.arange(P)[:, None]
        qq = np.arange(TQ)[None, :]
        masks[:, j * TQ:(j + 1) * TQ] = (j * P + kk <= qq).astype(np.float32)
    masks = masks.astype(ml_dtypes.bfloat16)
    ident = np.eye(P, dtype=np.float32)
    ones = np.ones((1, HD), dtype=np.float32)
    vpad = np.tile(np.array([1.0, 0.0], dtype=np.float32),
                   (P, BT // P)).astype(ml_dtypes.bfloat16)
    # head-major row permutation for wp: new row (h*8+j)*64+d = old (j*4+h)*64+d
    perm = np.array([(j * G + h) * HD + d
                     for h in range(G) for j in range(NCORES)
                     for d in range(HD)])
    wpp = np.ascontiguousarray(
        np.asarray(wp, np.float32)[perm, :]).astype(ml_dtypes.bfloat16)
    bpb = np.tile(np.asarray(bp, np.float32)[None, :], (P, 1))
    in_maps = []
    for i in range(NCORES):
        cs = slice(i * CQ, (i + 1) * CQ)
        ks = slice(i * HD, (i + 1) * HD)
        wkv = np.concatenate(
            [np.asarray(wk)[:, ks], np.asarray(wv)[:, ks]], axis=1)
        in_maps.append({
            "xt": xt,
            "wq": np.ascontiguousarray(
                np.asarray(wq, np.float32)[:, cs]).astype(ml_dtypes.bfloat16),
            "wkv": np.ascontiguousarray(
                wkv.astype(np.float32)).astype(ml_dtypes.bfloat16),
            "wpp": wpp,
            "bpb": bpb,
            "masks": masks,
            "ident": ident,
            "ones": ones,
            "vpad": vpad,
        })
    return in_maps


def kernel(x, wq, wk, wv, wp, bp, _trace=False):
    from concourse.bass_utils import run_bass_kernel_spmd
    nc = _get_nc()
    in_maps = make_in_maps(x, wq, wk, wv, wp, bp)
    res = run_bass_kernel_spmd(nc, in_maps, list(range(NCORES)), trace=_trace)
    out = np.concatenate([res.results[i]["out"] for i in range(NCORES)], axis=0)
    out = out.reshape(B, T, C).astype(np.float32)
    if _trace:
        return out, res
    return out


# revision 32
# speedup vs baseline: 5.1462x; 1.2316x over previous
"""GQA causal attention (B=2, T=2048, C=2048, 32 Q heads, 8 KV heads) on 8
Trainium2 NeuronCores.

Sharding: tensor-parallel attention over KV-head groups (core i owns KV head
i and its 4 query heads), then TOKEN-parallel output projection: instead of
AllGather-ing the full [C, BT] attention output (16MB collective), each core
AllToAll-exchanges normalized per-head outputs so core i ends up with
yT[all 2048 channels, its 512 tokens] (2MB collective, 8x less traffic) and
computes out[512 tokens, 2048 channels] with the full wp.

The A2A is staged per q-head (4 stages x 0.5MB) and overlaps attention
compute of the following head; the final projection starts on the first 3
stages' channels while the last A2A is still in flight.

Layouts/dtypes:
  - x fed pre-transposed as xt [C, BT] bf16; all matmul operands bf16
    (fp32 PSUM accumulate), so PE streams at full rate and DMA bytes halve.
  - scores computed transposed (s[k, q]) so softmax sum is a ones-column in
    the V matmul; V natural layout produced by PE-transposing vT blocks.
  - causal trimming: score/exp/yacc matmuls only cover the un-masked
    [off, TQ) column range of diagonal blocks.
  - wp is fed row-permuted (head-major) so each A2A stage's channels are
    contiguous contraction chunks.
"""

import sys

sys.path.insert(0, "/opt/trn_rl_repo")

import numpy as np
import ml_dtypes

import concourse.bass as bass
import concourse.mybir as mybir
import concourse.tile as tile

P = 128
B, T, C = 2, 2048, 2048
BT = B * T            # 4096
NH, NKV = 32, 8
HD = C // NH          # 64
G = NH // NKV         # 4 q heads per kv head / per core
CQ = G * HD           # 256 q channels per core
KC = C // P           # 16 contraction chunks
TQ = 512              # token-chunk
NCH = BT // TQ        # 8 token chunks == NCORES
NCORES = 8
NTOK = BT // NCORES   # 512 tokens per core for the output projection

f32 = mybir.dt.float32
f32r = mybir.dt.float32r
bf16 = mybir.dt.bfloat16
EXP = mybir.ActivationFunctionType.Exp
SCALE = float(HD) ** -0.5


def split_multi_waits(nc):
    """Walrus codegen allows only one sync-wait per engine instruction; move
    extras onto standalone same-engine EventSemaphore waits placed before."""
    for fn in nc.m.functions:
        for bb in fn.blocks:
            out = []
            for inst in bb.instructions:
                si = inst.sync_info
                if si is not None and si.on_wait and len(si.on_wait) > 1:
                    waits = list(si.on_wait)
                    for j, w in enumerate(waits[:-1]):
                        nop = mybir.InstEventSemaphore(
                            name=f"{inst.name}-ws{j}", ins=[], outs=[],
                            engine=inst.engine)
                        nop.sync_info = mybir.SyncInfo(on_wait=[w], on_update=[])
                        out.append(nop)
                    inst.sync_info = mybir.SyncInfo(
                        on_wait=[waits[-1]], on_update=list(si.on_update))
                out.append(inst)
            try:
                bb.instructions[:] = out
            except TypeError:
                bb.instructions.clear()
                bb.instructions.extend(out)


def build(n_rep=1):
    nc = bass.Bass(num_devices=NCORES)

    xt_d = nc.dram_tensor("xt", [C, BT], bf16, kind="ExternalInput")
    wq_d = nc.dram_tensor("wq", [C, CQ], bf16, kind="ExternalInput")
    wkv_d = nc.dram_tensor("wkv", [C, P], bf16, kind="ExternalInput")
    wpp_d = nc.dram_tensor("wpp", [C, C], bf16, kind="ExternalInput")
    bpb_d = nc.dram_tensor("bpb", [P, C], f32, kind="ExternalInput")
    mask_d = nc.dram_tensor("masks", [P, G * TQ], bf16, kind="ExternalInput")
    idn_d = nc.dram_tensor("ident", [P, P], f32, kind="ExternalInput")
    ones_d = nc.dram_tensor("ones", [1, HD], f32r, kind="ExternalInput")
    vpad_d = nc.dram_tensor("vpad", [P, (BT // P) * 2], bf16,
                            kind="ExternalInput")
    out_d = nc.dram_tensor("out", [NTOK, C], f32, kind="ExternalOutput")

    with tile.TileContext(nc) as tc:
      with tc.tile_pool(name="dram", bufs=1, space="DRAM") as dp:
        for rep in range(n_rep):
          with tc.tile_pool(name="act", bufs=1) as act:
            # phase-2/3 constants: tiles here, DMAs interleaved into phase 1
            # so the first xt/wq transfers aren't stuck behind them
            idn_sb = act.tile([P, P], f32)
            ones_sb = act.tile([1, HD], f32r)
            mask_sb = act.tile([P, G, TQ], bf16)
            bpb_sb = act.tile([P, C], f32)
            wp_sb = act.tile([P, KC, C], bf16)
            # long-lived activations; one qT tile per head so every matmul
            # operand sits at base partition 0
            qTh = [act.tile([HD, BT], bf16, name=f"qt{rep}_{h}")
                   for h in range(G)]
            kT = act.tile([HD, BT], bf16)
            va = act.tile([P, BT // P, HD + 2], bf16)  # v natural + ones col
            nc.sync.dma_start(
                va[:, :, HD:HD + 2],
                vpad_d.rearrange("p (k c) -> p k c", c=2))
            yt_sb = act.tile([P, KC, TQ], bf16)  # A2A result (proj lhsT)
            # A2A stages {h0,h1}, {h2}, {h3}: one contiguous dest-major DRAM
            # tile per stage so a stage's collective read doesn't falsely
            # overlap later heads' writes
            STAGES = ((0, 2), (2, 3), (3, 4))
            st_loc = [dp.tile([NCORES, hi - lo, HD, TQ], bf16,
                              name=f"stl{rep}_{si}")
                      for si, (lo, hi) in enumerate(STAGES)]
            yt_a2a = dp.tile([C, TQ], bf16)

            # ---- Phase 1: q/k/v projections (contract C on partitions) ----
            with tc.tile_pool(name="xp", bufs=4) as xp, \
                 tc.tile_pool(name="w1", bufs=1) as w1, \
                 tc.tile_pool(name="pps", bufs=2, space="PSUM") as pps, \
                 tc.tile_pool(name="tps", bufs=2, space="PSUM") as tps:
                wq_sb = w1.tile([P, KC, CQ], bf16)
                nc.sync.dma_start(wq_sb[:], wq_d.rearrange("(o p) n -> p o n", p=P))
                wkv_sb = w1.tile([P, KC, P], bf16)
                nc.sync.dma_start(wkv_sb[:], wkv_d.rearrange("(o p) n -> p o n", p=P))
                # idn is consumed by tb=0's transposes: must be written first
                nc.sync.dma_start(idn_sb[:], idn_d[:, :])
                nc.sync.dma_start(ones_sb[:], ones_d[:, :])
                for tb in range(BT // TQ):
                    if tb == 1:
                        nc.sync.dma_start(
                            mask_sb[:],
                            mask_d.rearrange("p (g t) -> p g t", g=G))
                        nc.sync.dma_start(bpb_sb[:], bpb_d[:, :])
                    elif tb >= 2:
                        # two 0.5MB wp chunks per tb: c0-11 by tb=7, rest after
                        for w2 in range(2):
                            cw = (tb - 2) * 2 + w2
                            nc.sync.dma_start(
                                wp_sb[:, cw, :], wpp_d[cw * P:(cw + 1) * P, :])
                    q0_ps = pps.tile([P, TQ], f32, tag="q0")
                    q1_ps = pps.tile([P, TQ], f32, tag="q1")
                    kv_ps = pps.tile([P, TQ], f32, tag="kv")
                    for cg in range(KC // 4):
                        xt_t = xp.tile([P, 4, TQ], bf16, tag="xt")
                        nc.sync.dma_start(
                            xt_t[:],
                            xt_d[cg * 4 * P:(cg + 1) * 4 * P,
                                 tb * TQ:(tb + 1) * TQ].rearrange(
                                "(o p) n -> p o n", p=P))
                        for cc in range(4):
                            c = cg * 4 + cc
                            nc.tensor.matmul(q0_ps[:], wq_sb[:, c, 0:P],
                                             xt_t[:, cc, :],
                                             start=(c == 0), stop=(c == KC - 1))
                            nc.tensor.matmul(q1_ps[:], wq_sb[:, c, P:CQ],
                                             xt_t[:, cc, :],
                                             start=(c == 0), stop=(c == KC - 1))
                            nc.tensor.matmul(kv_ps[:], wkv_sb[:, c, :],
                                             xt_t[:, cc, :],
                                             start=(c == 0), stop=(c == KC - 1))
                    sl = slice(tb * TQ, (tb + 1) * TQ)
                    nc.vector.tensor_copy(qTh[0][:, sl], q0_ps[0:HD, :])
                    nc.vector.tensor_copy(qTh[1][:, sl], q0_ps[HD:P, :])
                    nc.vector.tensor_copy(qTh[2][:, sl], q1_ps[0:HD, :])
                    nc.vector.tensor_copy(qTh[3][:, sl], q1_ps[HD:P, :])
                    nc.vector.tensor_copy(kT[:, sl], kv_ps[0:HD, :])
                    vs_t = xp.tile([HD, TQ], f32, tag="vs")
                    nc.vector.tensor_copy(vs_t[:], kv_ps[HD:P, :])
                    # V natural layout via PE transpose of vT blocks
                    for k4 in range(TQ // P):
                        kb = tb * (TQ // P) + k4
                        vt_ps = tps.tile([P, HD], f32, tag="vt")
                        nc.tensor.transpose(vt_ps[:], vs_t[:, k4 * P:(k4 + 1) * P],
                                            idn_sb[0:HD, 0:HD])
                        nc.vector.tensor_copy(va[:, kb, 0:HD], vt_ps[:])
                for cw in range(12, KC):
                    nc.sync.dma_start(
                        wp_sb[:, cw, :], wpp_d[cw * P:(cw + 1) * P, :])

            # ---- Phase 2: causal attention (scores-transposed, trimmed),
            # head-staged normalize + AllToAll overlapped with next head ----
            with tc.tile_pool(name="aps", bufs=2, space="PSUM") as aps, \
                 tc.tile_pool(name="yps", bufs=2, space="PSUM") as yps, \
                 tc.tile_pool(name="bps", bufs=2, space="PSUM") as bps, \
                 tc.tile_pool(name="ep", bufs=6) as ep, \
                 tc.tile_pool(name="np_", bufs=4) as npo:
                for qh in range(G):
                    for ci in range(NCH):
                        b, qc = ci // (T // TQ), ci % (T // TQ)
                        nkb = 4 * qc + 4
                        y_ps = yps.tile([HD + 2, TQ], f32, tag="y")
                        qap = qTh[qh][:, b * T + qc * TQ: b * T + (qc + 1) * TQ]
                        for kbp in range(nkb // 2):
                            j0 = 2 * kbp - 4 * qc
                            offp = P * max(0, j0)
                            s_ps = aps.tile([P, 2, TQ], f32, tag="s")
                            for h in range(2):
                                kb = 2 * kbp + h
                                off = P * max(0, kb - 4 * qc)
                                nc.tensor.matmul(
                                    s_ps[:, h, off:],
                                    kT[:, b * T + kb * P: b * T + (kb + 1) * P],
                                    qap[:, off:], start=True, stop=True)
                            ex = ep.tile([P, 2, TQ], bf16, tag="ex")
                            nc.scalar.activation(ex[:, :, offp:],
                                                 s_ps[:, :, offp:], EXP,
                                                 scale=SCALE)
                            for h in range(2):
                                kb = 2 * kbp + h
                                j = kb - 4 * qc
                                off = P * max(0, j)
                                if j >= 0:
                                    nc.vector.tensor_mul(
                                        ex[:, h, off:off + P],
                                        ex[:, h, off:off + P],
                                        mask_sb[:, j, off:off + P])
                                nc.tensor.matmul(
                                    y_ps[:, off:],
                                    va[:, b * (T // P) + kb, :],
                                    ex[:, h, off:],
                                    start=(kb == 0), stop=(kb == nkb - 1),
                                    skip_group_check=True)
                        # normalize this chunk right away: r = 1/l broadcast
                        # down the 64 dims via ones-matmul (gpsimd is owned
                        # by the collectives, so the broadcast rides on PE)
                        rrow = npo.tile([1, TQ], f32r, tag="rr", bufs=2)
                        with nc.allow_low_precision(
                                reason="1/l as f32r matmul operand (~5e-6)"):
                            nc.vector.reciprocal(rrow[:], y_ps[HD:HD + 1, :])
                        ystage = npo.tile([HD, TQ], bf16, tag="ys", bufs=2)
                        nc.vector.tensor_copy(ystage[:], y_ps[0:HD, :])
                        bc_ps = bps.tile([HD, TQ], f32, tag="bc")
                        nc.tensor.matmul(bc_ps[:], ones_sb[:], rrow[:],
                                         start=True, stop=True)
                        yn = npo.tile([HD, TQ], bf16, tag="yn")
                        nc.vector.tensor_mul(yn[:], ystage[:], bc_ps[:])
                        si = 0 if qh < 2 else qh - 1
                        nc.sync.dma_start(
                            st_loc[si][ci, qh - STAGES[si][0], :, :], yn[:])
                    si = 0 if qh < 2 else qh - 1
                    lo, hi = STAGES[si]
                    if qh == hi - 1:  # stage complete: ship it
                        r0, r1 = lo * NCORES * HD, hi * NCORES * HD
                        nc.gpsimd.collective_compute(
                            "AllToAll", mybir.AluOpType.bypass,
                            replica_groups=[list(range(NCORES))],
                            ins=[st_loc[si][:].opt()],
                            outs=[yt_a2a[r0:r1, :].opt()])
                # land stage channels for the projection, on the gpsimd queue
                # (in-order behind the last A2A, so no engine stalls on the
                # completion wait)
                for (r0, r1) in ((0, 1024), (1024, 1536), (1536, 2048)):
                    nc.gpsimd.dma_start(
                        yt_sb[:, r0 // P:r1 // P, :],
                        yt_a2a[r0:r1, :].rearrange("(c p) t -> p c t", p=P))

            # ---- Phase 3: token-sharded output projection + bias ----
            # Column-groups of 512 out-channels x all 4 token blocks (4 PSUM
            # banks each, 2 groups in flight). All groups pre-accumulate the
            # A2A stage-0..2 channels (c0-11) while the last A2A is in
            # flight; groups 0/1 park their partials in SBUF and preload
            # them back into PSUM for the c12-15 finish.
            NG = C // TQ
            with tc.tile_pool(name="fps", bufs=2, space="PSUM") as fps, \
                 tc.tile_pool(name="fp", bufs=2) as fp:
                keep = {}
                dumps = {}
                CPRE = 12  # chunks from A2A stages 0-2

                def proj_chunks(ops, g, c0, c1, start):
                    osl = slice(g * TQ, (g + 1) * TQ)
                    for c in range(c0, c1):
                        for i in range(4):
                            nc.tensor.matmul(
                                ops[i][:], yt_sb[:, c, i * P:(i + 1) * P],
                                wp_sb[:, c, osl],
                                start=start and (c == c0), stop=(c == c1 - 1),
                                skip_group_check=True)

                for g in range(NG):
                    ops = [fps.tile([P, TQ], f32, tag=f"t{i}",
                                    name=f"o{rep}_{g}_{i}") for i in range(4)]
                    proj_chunks(ops, g, 0, CPRE, start=True)
                    if g < 2:
                        dsb = fp.tile([P, 4, TQ], f32, tag=f"d{g}", bufs=1,
                                      name=f"d{rep}_{g}")
                        for i in range(4):
                            nc.vector.tensor_copy(dsb[:, i, :], ops[i][:])
                        dumps[g] = dsb
                    else:
                        keep[g] = ops
                for g in (2, 3, 0, 1):
                    osl = slice(g * TQ, (g + 1) * TQ)
                    if g < 2:
                        ops = [fps.tile([P, TQ], f32, tag=f"t{i}",
                                        name=f"p{rep}_{g}_{i}")
                               for i in range(4)]
                        for i in range(4):
                            nc.vector.tensor_copy(ops[i][:], dumps[g][:, i, :])
                    else:
                        ops = keep[g]
                    proj_chunks(ops, g, CPRE, KC, start=False)
                    for i in range(4):
                        o_sb = fp.tile([P, TQ], f32, tag="ob", bufs=4)
                        nc.vector.tensor_add(o_sb[:], ops[i][:],
                                             bpb_sb[:, osl])
                        nc.sync.dma_start(
                            out_d[i * P:(i + 1) * P, osl], o_sb[:])

    split_multi_waits(nc)
    return nc


_NC_CACHE = {}


def _get_nc(n_rep=1):
    if n_rep not in _NC_CACHE:
        _NC_CACHE[n_rep] = build(n_rep)
    return _NC_CACHE[n_rep]


def make_in_maps(x, wq, wk, wv, wp, bp):
    x = np.asarray(x, dtype=np.float32)
    xt = np.ascontiguousarray(x.reshape(BT, C).T).astype(ml_dtypes.bfloat16)
    masks = np.zeros((P, G * TQ), dtype=np.float32)
    for j in range(G):
        kk = np.arange(P)[:, None]
        qq = np.arange(TQ)[None, :]
        masks[:, j * TQ:(j + 1) * TQ] = (j * P + kk <= qq).astype(np.float32)
    masks = masks.astype(ml_dtypes.bfloat16)
    ident = np.eye(P, dtype=np.float32)
    ones = np.ones((1, HD), dtype=np.float32)
    vpad = np.tile(np.array([1.0, 0.0], dtype=np.float32),
                   (P, BT // P)).astype(ml_dtypes.bfloat16)
    # wp row permutation matching the A2A arrival order:
    # stage {h0,h1}: (src core j, h in {0,1}, d); stages {h2}, {h3}: (j, d)
    perm = ([(j * G + h) * HD + d
             for j in range(NCORES) for h in range(2) for d in range(HD)] +
            [(j * G + h) * HD + d
             for h in (2, 3) for j in range(NCORES) for d in range(HD)])
    perm = np.array(perm)
    wpp = np.ascontiguousarray(
        np.asarray(wp, np.float32)[perm, :]).astype(ml_dtypes.bfloat16)
    bpb = np.tile(np.asarray(bp, np.float32)[None, :], (P, 1))
    in_maps = []
    for i in range(NCORES):
        cs = slice(i * CQ, (i + 1) * CQ)
        ks = slice(i * HD, (i + 1) * HD)
        wkv = np.concatenate(
            [np.asarray(wk)[:, ks], np.asarray(wv)[:, ks]], axis=1)
        in_maps.append({
            "xt": xt,
            "wq": np.ascontiguousarray(
                np.asarray(wq, np.float32)[:, cs]).astype(ml_dtypes.bfloat16),
            "wkv": np.ascontiguousarray(
                wkv.astype(np.float32)).astype(ml_dtypes.bfloat16),
            "wpp": wpp,
            "bpb": bpb,
            "masks": masks,
            "ident": ident,
            "ones": ones,
            "vpad": vpad,
        })
    return in_maps


def kernel(x, wq, wk, wv, wp, bp, _trace=False):
    from concourse.bass_utils import run_bass_kernel_spmd
    nc = _get_nc()
    in_maps = make_in_maps(x, wq, wk, wv, wp, bp)
    res = run_bass_kernel_spmd(nc, in_maps, list(range(NCORES)), trace=_trace)
    out = np.concatenate([res.results[i]["out"] for i in range(NCORES)], axis=0)
    out = out.reshape(B, T, C).astype(np.float32)
    if _trace:
        return out, res
    return out


# revision 40
# speedup vs baseline: 5.5940x; 1.0870x over previous
"""GQA causal attention (B=2, T=2048, C=2048, 32 Q heads, 8 KV heads) on 8
Trainium2 NeuronCores.

Sharding: tensor-parallel attention over KV-head groups (core i owns KV head
i and its 4 query heads), then TOKEN-parallel output projection: instead of
AllGather-ing the full [C, BT] attention output (16MB collective), each core
AllToAll-exchanges normalized per-head outputs so core i ends up with
yT[all 2048 channels, its 512 tokens] (2MB collective, 8x less traffic) and
computes out[512 tokens, 2048 channels] with the full wp.

The A2A is staged per q-head (4 stages x 0.5MB) and overlaps attention
compute of the following head; the final projection starts on the first 3
stages' channels while the last A2A is still in flight.

Layouts/dtypes:
  - x fed pre-transposed as xt [C, BT] bf16; all matmul operands bf16
    (fp32 PSUM accumulate), so PE streams at full rate and DMA bytes halve.
  - scores computed transposed (s[k, q]) so softmax sum is a ones-column in
    the V matmul; V natural layout produced by PE-transposing vT blocks.
  - causal trimming: score/exp/yacc matmuls only cover the un-masked
    [off, TQ) column range of diagonal blocks.
  - wp is fed row-permuted (head-major) so each A2A stage's channels are
    contiguous contraction chunks.
"""

import sys

sys.path.insert(0, "/opt/trn_rl_repo")

import numpy as np
import ml_dtypes

import concourse.bass as bass
import concourse.mybir as mybir
import concourse.tile as tile

P = 128
B, T, C = 2, 2048, 2048
BT = B * T            # 4096
NH, NKV = 32, 8
HD = C // NH          # 64
G = NH // NKV         # 4 q heads per kv head / per core
CQ = G * HD           # 256 q channels per core
KC = C // P           # 16 contraction chunks
TQ = 512              # token-chunk
NCH = BT // TQ        # 8 token chunks == NCORES
NCORES = 8
NTOK = BT // NCORES   # 512 tokens per core for the output projection

f32 = mybir.dt.float32
f32r = mybir.dt.float32r
bf16 = mybir.dt.bfloat16
EXP = mybir.ActivationFunctionType.Exp
SCALE = float(HD) ** -0.5


def split_multi_waits(nc):
    """Walrus codegen allows only one sync-wait per engine instruction; move
    extras onto standalone same-engine EventSemaphore waits placed before."""
    for fn in nc.m.functions:
        for bb in fn.blocks:
            out = []
            for inst in bb.instructions:
                si = inst.sync_info
                if si is not None and si.on_wait and len(si.on_wait) > 1:
                    waits = list(si.on_wait)
                    for j, w in enumerate(waits[:-1]):
                        nop = mybir.InstEventSemaphore(
                            name=f"{inst.name}-ws{j}", ins=[], outs=[],
                            engine=inst.engine)
                        nop.sync_info = mybir.SyncInfo(on_wait=[w], on_update=[])
                        out.append(nop)
                    inst.sync_info = mybir.SyncInfo(
                        on_wait=[waits[-1]], on_update=list(si.on_update))
                out.append(inst)
            try:
                bb.instructions[:] = out
            except TypeError:
                bb.instructions.clear()
                bb.instructions.extend(out)


def build(n_rep=1):
    nc = bass.Bass(num_devices=NCORES)

    xt_d = nc.dram_tensor("xt", [C, BT], bf16, kind="ExternalInput")
    wq_d = nc.dram_tensor("wq", [C, CQ], bf16, kind="ExternalInput")
    wkv_d = nc.dram_tensor("wkv", [C, P], bf16, kind="ExternalInput")
    wpp_d = nc.dram_tensor("wpp", [C, C], bf16, kind="ExternalInput")
    bpb_d = nc.dram_tensor("bpb", [P, C], f32, kind="ExternalInput")
    mask_d = nc.dram_tensor("masks", [P, G * TQ], bf16, kind="ExternalInput")
    idn_d = nc.dram_tensor("ident", [P, P], f32, kind="ExternalInput")
    ones_d = nc.dram_tensor("ones", [1, HD], f32r, kind="ExternalInput")
    vpad_d = nc.dram_tensor("vpad", [P, (BT // P) * 2], bf16,
                            kind="ExternalInput")
    out_d = nc.dram_tensor("out", [NTOK, C], f32, kind="ExternalOutput")

    with tile.TileContext(nc) as tc:
      with tc.tile_pool(name="dram", bufs=1, space="DRAM") as dp:
        for rep in range(n_rep):
          with tc.tile_pool(name="act", bufs=1) as act:
            # phase-2/3 constants: tiles here, DMAs interleaved into phase 1
            # so the first xt/wq transfers aren't stuck behind them
            idn_sb = act.tile([P, P], f32)
            ones_sb = act.tile([1, HD], f32r)
            mask_sb = act.tile([P, G, TQ], bf16)
            bpb_sb = act.tile([P, C], f32)
            wp_sb = act.tile([P, KC, C], bf16)
            # long-lived activations; one qT tile per head so every matmul
            # operand sits at base partition 0
            qTh = [act.tile([HD, BT], bf16, name=f"qt{rep}_{h}")
                   for h in range(G)]
            kT = act.tile([HD, BT], bf16)
            va = act.tile([P, BT // P, HD + 2], bf16)  # v natural + ones col
            nc.sync.dma_start(
                va[:, :, HD:HD + 2],
                vpad_d.rearrange("p (k c) -> p k c", c=2))
            yt_sb = act.tile([P, KC, TQ], bf16)  # A2A result (proj lhsT)
            # A2A stages {h0,h1}, {h2}, {h3}: one contiguous dest-major DRAM
            # tile per stage so a stage's collective read doesn't falsely
            # overlap later heads' writes
            STAGES = ((0, 2), (2, 3), (3, 4))
            st_loc = [dp.tile([NCORES, hi - lo, HD, TQ], bf16,
                              name=f"stl{rep}_{si}")
                      for si, (lo, hi) in enumerate(STAGES)]
            yt_a2a = dp.tile([C, TQ], bf16)

            # ---- Phase 1: q/k/v projections (contract C on partitions) ----
            with tc.tile_pool(name="xp", bufs=4) as xp, \
                 tc.tile_pool(name="w1", bufs=1) as w1, \
                 tc.tile_pool(name="pps", bufs=2, space="PSUM") as pps, \
                 tc.tile_pool(name="tps", bufs=2, space="PSUM") as tps:
                # first c-chunks of wq/wkv land first so tb=0 starts sooner
                wq_sb = w1.tile([P, KC, CQ], bf16)
                wkv_sb = w1.tile([P, KC, P], bf16)
                for c0, c1 in ((0, 2), (2, KC)):
                    nc.sync.dma_start(
                        wq_sb[:, c0:c1, :],
                        wq_d[c0 * P:c1 * P, :].rearrange("(o p) n -> p o n", p=P))
                    nc.sync.dma_start(
                        wkv_sb[:, c0:c1, :],
                        wkv_d[c0 * P:c1 * P, :].rearrange("(o p) n -> p o n", p=P))
                    if c0 == 0:
                        # idn is consumed by tb=0's transposes
                        nc.sync.dma_start(idn_sb[:], idn_d[:, :])
                        nc.sync.dma_start(ones_sb[:], ones_d[:, :])
                vs_q = []
                for tb in range(BT // TQ):
                    if tb == 1:
                        nc.sync.dma_start(
                            mask_sb[:],
                            mask_d.rearrange("p (g t) -> p g t", g=G))
                        nc.sync.dma_start(bpb_sb[:], bpb_d[:, :])
                    elif tb >= 2:
                        # two 0.5MB wp chunks per tb: c0-11 by tb=7, rest after
                        for w2 in range(2):
                            cw = (tb - 2) * 2 + w2
                            nc.sync.dma_start(
                                wp_sb[:, cw, :], wpp_d[cw * P:(cw + 1) * P, :])
                    q0_ps = pps.tile([P, TQ], f32, tag="q0")
                    q1_ps = pps.tile([P, TQ], f32, tag="q1")
                    kv_ps = pps.tile([P, TQ], f32, tag="kv")
                    for cg in range(KC // 4):
                        xt_t = xp.tile([P, 4, TQ], bf16, tag="xt")
                        nc.sync.dma_start(
                            xt_t[:],
                            xt_d[cg * 4 * P:(cg + 1) * 4 * P,
                                 tb * TQ:(tb + 1) * TQ].rearrange(
                                "(o p) n -> p o n", p=P))
                        for cc in range(4):
                            c = cg * 4 + cc
                            nc.tensor.matmul(q0_ps[:], wq_sb[:, c, 0:P],
                                             xt_t[:, cc, :],
                                             start=(c == 0), stop=(c == KC - 1))
                            nc.tensor.matmul(q1_ps[:], wq_sb[:, c, P:CQ],
                                             xt_t[:, cc, :],
                                             start=(c == 0), stop=(c == KC - 1))
                            nc.tensor.matmul(kv_ps[:], wkv_sb[:, c, :],
                                             xt_t[:, cc, :],
                                             start=(c == 0), stop=(c == KC - 1))
                    sl = slice(tb * TQ, (tb + 1) * TQ)
                    nc.vector.tensor_copy(qTh[0][:, sl], q0_ps[0:HD, :])
                    nc.vector.tensor_copy(qTh[1][:, sl], q0_ps[HD:P, :])
                    nc.vector.tensor_copy(qTh[2][:, sl], q1_ps[0:HD, :])
                    nc.vector.tensor_copy(qTh[3][:, sl], q1_ps[HD:P, :])
                    nc.vector.tensor_copy(kT[:, sl], kv_ps[0:HD, :])
                    vs_t = xp.tile([HD, TQ], f32, tag="vs")
                    nc.vector.tensor_copy(vs_t[:], kv_ps[HD:P, :])
                    # V transposes deferred one tb so they overlap the next
                    # tb's matmuls instead of stalling PE on the vs copy
                    vs_q.append((tb, vs_t))
                    if len(vs_q) == 2:
                        dtb, dvs = vs_q.pop(0)
                        for k4 in range(TQ // P):
                            kb = dtb * (TQ // P) + k4
                            vt_ps = tps.tile([P, HD], f32, tag="vt")
                            nc.tensor.transpose(vt_ps[:],
                                                dvs[:, k4 * P:(k4 + 1) * P],
                                                idn_sb[0:HD, 0:HD])
                            nc.vector.tensor_copy(va[:, kb, 0:HD], vt_ps[:])
                dtb, dvs = vs_q.pop(0)
                for k4 in range(TQ // P):
                    kb = dtb * (TQ // P) + k4
                    vt_ps = tps.tile([P, HD], f32, tag="vt")
                    nc.tensor.transpose(vt_ps[:], dvs[:, k4 * P:(k4 + 1) * P],
                                        idn_sb[0:HD, 0:HD])
                    nc.vector.tensor_copy(va[:, kb, 0:HD], vt_ps[:])
                for cw in range(12, KC):
                    nc.sync.dma_start(
                        wp_sb[:, cw, :], wpp_d[cw * P:(cw + 1) * P, :])

            # ---- Phase 2: causal attention (scores-transposed, trimmed),
            # head-staged normalize + AllToAll overlapped with next head ----
            with tc.tile_pool(name="aps", bufs=2, space="PSUM") as aps, \
                 tc.tile_pool(name="yps", bufs=2, space="PSUM") as yps, \
                 tc.tile_pool(name="bps", bufs=2, space="PSUM") as bps, \
                 tc.tile_pool(name="ep", bufs=6) as ep, \
                 tc.tile_pool(name="np_", bufs=4) as npo:
                def emit_y(pend):
                    """mask + V-accumulate for a finished score pair."""
                    y_ps, ex, b, qc, kbp, nkb = pend
                    for h in range(2):
                        kb = 2 * kbp + h
                        j = kb - 4 * qc
                        off = P * max(0, j)
                        if j >= 0:
                            nc.vector.tensor_mul(
                                ex[:, h, off:off + P],
                                ex[:, h, off:off + P],
                                mask_sb[:, j, off:off + P])
                        nc.tensor.matmul(
                            y_ps[:, off:],
                            va[:, b * (T // P) + kb, :],
                            ex[:, h, off:],
                            start=(kb == 0), stop=(kb == nkb - 1),
                            skip_group_check=True)

                def emit_norm(pend):
                    """r = 1/l, ones-matmul broadcast, scale, ship to stage."""
                    y_ps, qh, ci = pend
                    rrow = npo.tile([1, TQ], f32r, tag="rr", bufs=2)
                    with nc.allow_low_precision(
                            reason="1/l as f32r matmul operand (~5e-6)"):
                        nc.vector.reciprocal(rrow[:], y_ps[HD:HD + 1, :])
                    ystage = npo.tile([HD, TQ], bf16, tag="ys", bufs=2)
                    nc.vector.tensor_copy(ystage[:], y_ps[0:HD, :])
                    bc_ps = bps.tile([HD, TQ], f32, tag="bc")
                    nc.tensor.matmul(bc_ps[:], ones_sb[:], rrow[:],
                                     start=True, stop=True)
                    yn = npo.tile([HD, TQ], bf16, tag="yn")
                    nc.vector.tensor_mul(yn[:], ystage[:], bc_ps[:])
                    si = 0 if qh < 2 else qh - 1
                    nc.sync.dma_start(
                        st_loc[si][ci, qh - STAGES[si][0], :, :], yn[:])

                # software-pipelined: score matmuls run one pair ahead of the
                # V-accumulates (and the previous chunk's normalize slots in
                # after the first pair) so the in-order PE queue never waits
                # on an exp that hasn't finished
                pend_y = None
                pend_norm = None
                for qh in range(G):
                    for ci in range(NCH):
                        b, qc = ci // (T // TQ), ci % (T // TQ)
                        nkb = 4 * qc + 4
                        y_ps = yps.tile([HD + 2, TQ], f32, tag="y")
                        qap = qTh[qh][:, b * T + qc * TQ: b * T + (qc + 1) * TQ]
                        for kbp in range(nkb // 2):
                            j0 = 2 * kbp - 4 * qc
                            offp = P * max(0, j0)
                            s_ps = aps.tile([P, 2, TQ], f32, tag="s")
                            for h in range(2):
                                kb = 2 * kbp + h
                                off = P * max(0, kb - 4 * qc)
                                nc.tensor.matmul(
                                    s_ps[:, h, off:],
                                    kT[:, b * T + kb * P: b * T + (kb + 1) * P],
                                    qap[:, off:], start=True, stop=True)
                            ex = ep.tile([P, 2, TQ], bf16, tag="ex")
                            nc.scalar.activation(ex[:, :, offp:],
                                                 s_ps[:, :, offp:], EXP,
                                                 scale=SCALE)
                            if pend_y is not None:
                                emit_y(pend_y)
                            pend_y = (y_ps, ex, b, qc, kbp, nkb)
                            if pend_norm is not None:
                                emit_norm(pend_norm)
                                pend_norm = None
                        pend_norm = (y_ps, qh, ci)
                    # flush before the head's A2A: its stage needs all chunks
                    si = 0 if qh < 2 else qh - 1
                    lo, hi = STAGES[si]
                    if qh == hi - 1:  # stage complete: ship it
                        if pend_y is not None:
                            emit_y(pend_y)
                            pend_y = None
                        if pend_norm is not None:
                            emit_norm(pend_norm)
                            pend_norm = None
                        r0, r1 = lo * NCORES * HD, hi * NCORES * HD
                        nc.gpsimd.collective_compute(
                            "AllToAll", mybir.AluOpType.bypass,
                            replica_groups=[list(range(NCORES))],
                            ins=[st_loc[si][:].opt()],
                            outs=[yt_a2a[r0:r1, :].opt()])
                # land stage channels for the projection, on the gpsimd queue
                # (in-order behind the last A2A, so no engine stalls on the
                # completion wait)
                for (r0, r1) in ((0, 1024), (1024, 1536), (1536, 2048)):
                    nc.gpsimd.dma_start(
                        yt_sb[:, r0 // P:r1 // P, :],
                        yt_a2a[r0:r1, :].rearrange("(c p) t -> p c t", p=P))

            # ---- Phase 3: token-sharded output projection + bias ----
            # Column-groups of 512 out-channels x all 4 token blocks (4 PSUM
            # banks each, 2 groups in flight). All groups pre-accumulate the
            # A2A stage-0..2 channels (c0-11) while the last A2A is in
            # flight; groups 0/1 park their partials in SBUF and preload
            # them back into PSUM for the c12-15 finish.
            NG = C // TQ
            with tc.tile_pool(name="fps", bufs=2, space="PSUM") as fps, \
                 tc.tile_pool(name="fp", bufs=2) as fp:
                keep = {}
                dumps = {}
                CPRE = 12  # chunks from A2A stages 0-2

                def proj_chunks(ops, g, c0, c1, start):
                    osl = slice(g * TQ, (g + 1) * TQ)
                    for c in range(c0, c1):
                        for i in range(4):
                            nc.tensor.matmul(
                                ops[i][:], yt_sb[:, c, i * P:(i + 1) * P],
                                wp_sb[:, c, osl],
                                start=start and (c == c0), stop=(c == c1 - 1),
                                skip_group_check=True)

                for g in range(NG):
                    ops = [fps.tile([P, TQ], f32, tag=f"t{i}",
                                    name=f"o{rep}_{g}_{i}") for i in range(4)]
                    proj_chunks(ops, g, 0, CPRE, start=True)
                    if g < 2:
                        dsb = fp.tile([P, 4, TQ], f32, tag=f"d{g}", bufs=1,
                                      name=f"d{rep}_{g}")
                        for i in range(4):
                            nc.vector.tensor_copy(dsb[:, i, :], ops[i][:])
                        dumps[g] = dsb
                    else:
                        keep[g] = ops
                for g in (2, 3, 0, 1):
                    osl = slice(g * TQ, (g + 1) * TQ)
                    if g < 2:
                        ops = [fps.tile([P, TQ], f32, tag=f"t{i}",
                                        name=f"p{rep}_{g}_{i}")
                               for i in range(4)]
                        for i in range(4):
                            nc.vector.tensor_copy(ops[i][:], dumps[g][:, i, :])
                    else:
                        ops = keep[g]
                    proj_chunks(ops, g, CPRE, KC, start=False)
                    for i in range(4):
                        o_sb = fp.tile([P, TQ], f32, tag="ob", bufs=4)
                        nc.vector.tensor_add(o_sb[:], ops[i][:],
                                             bpb_sb[:, osl])
                        nc.sync.dma_start(
                            out_d[i * P:(i + 1) * P, osl], o_sb[:])

    split_multi_waits(nc)
    return nc


_NC_CACHE = {}


def _get_nc(n_rep=1):
    if n_rep not in _NC_CACHE:
        _NC_CACHE[n_rep] = build(n_rep)
    return _NC_CACHE[n_rep]


def make_in_maps(x, wq, wk, wv, wp, bp):
    x = np.asarray(x, dtype=np.float32)
    xt = np.ascontiguousarray(x.reshape(BT, C).T).astype(ml_dtypes.bfloat16)
    masks = np.zeros((P, G * TQ), dtype=np.float32)
    for j in range(G):
        kk = np.arange(P)[:, None]
        qq = np.arange(TQ)[None, :]
        masks[:, j * TQ:(j + 1) * TQ] = (j * P + kk <= qq).astype(np.float32)
    masks = masks.astype(ml_dtypes.bfloat16)
    ident = np.eye(P, dtype=np.float32)
    ones = np.ones((1, HD), dtype=np.float32)
    vpad = np.tile(np.array([1.0, 0.0], dtype=np.float32),
                   (P, BT // P)).astype(ml_dtypes.bfloat16)
    # wp row permutation matching the A2A arrival order:
    # stage {h0,h1}: (src core j, h in {0,1}, d); stages {h2}, {h3}: (j, d)
    perm = ([(j * G + h) * HD + d
             for j in range(NCORES) for h in range(2) for d in range(HD)] +
            [(j * G + h) * HD + d
             for h in (2, 3) for j in range(NCORES) for d in range(HD)])
    perm = np.array(perm)
    wpp = np.ascontiguousarray(
        np.asarray(wp, np.float32)[perm, :]).astype(ml_dtypes.bfloat16)
    bpb = np.tile(np.asarray(bp, np.float32)[None, :], (P, 1))
    in_maps = []
    for i in range(NCORES):
        cs = slice(i * CQ, (i + 1) * CQ)
        ks = slice(i * HD, (i + 1) * HD)
        wkv = np.concatenate(
            [np.asarray(wk)[:, ks], np.asarray(wv)[:, ks]], axis=1)
        in_maps.append({
            "xt": xt,
            "wq": np.ascontiguousarray(
                np.asarray(wq, np.float32)[:, cs]).astype(ml_dtypes.bfloat16),
            "wkv": np.ascontiguousarray(
                wkv.astype(np.float32)).astype(ml_dtypes.bfloat16),
            "wpp": wpp,
            "bpb": bpb,
            "masks": masks,
            "ident": ident,
            "ones": ones,
            "vpad": vpad,
        })
    return in_maps


def kernel(x, wq, wk, wv, wp, bp, _trace=False):
    from concourse.bass_utils import run_bass_kernel_spmd
    nc = _get_nc()
    in_maps = make_in_maps(x, wq, wk, wv, wp, bp)
    res = run_bass_kernel_spmd(nc, in_maps, list(range(NCORES)), trace=_trace)
    out = np.concatenate([res.results[i]["out"] for i in range(NCORES)], axis=0)
    out = out.reshape(B, T, C).astype(np.float32)
    if _trace:
        return out, res
    return out


# revision 51
# speedup vs baseline: 5.6786x; 1.0151x over previous
"""GQA causal attention (B=2, T=2048, C=2048, 32 Q heads, 8 KV heads) on 8
Trainium2 NeuronCores.

Sharding: tensor-parallel attention over KV-head groups (core i owns KV head
i and its 4 query heads), then TOKEN-parallel output projection: instead of
AllGather-ing the full [C, BT] attention output (16MB collective), each core
AllToAll-exchanges normalized per-head outputs so core i ends up with
yT[all 2048 channels, its 512 tokens] (2MB collective, 8x less traffic) and
computes out[512 tokens, 2048 channels] with the full wp.

The A2A is staged {h0,h1}, {h2}, {h3} so each stage overlaps attention
compute of the following head and the cheap single-head stage comes last;
the final projection pre-accumulates the first two stages' channels for all
four token blocks while the last A2A is still in flight.

Layouts/dtypes:
  - x fed pre-transposed as xt [C, BT] bf16; all matmul operands bf16
    (fp32 PSUM accumulate), so PE streams at full rate and DMA bytes halve.
  - scores computed transposed (s[k, q]) so softmax sum is a ones-column in
    the V matmul; V natural layout produced by PE-transposing vT blocks.
  - causal trimming: score/exp/yacc matmuls only cover the un-masked
    [off, TQ) column range of diagonal blocks.
  - wp is fed row-permuted (head-major) so each A2A stage's channels are
    contiguous contraction chunks.
"""

import sys

sys.path.insert(0, "/opt/trn_rl_repo")

import numpy as np
import ml_dtypes

import concourse.bass as bass
import concourse.mybir as mybir
import concourse.tile as tile

P = 128
B, T, C = 2, 2048, 2048
BT = B * T            # 4096
NH, NKV = 32, 8
HD = C // NH          # 64
G = NH // NKV         # 4 q heads per kv head / per core
CQ = G * HD           # 256 q channels per core
KC = C // P           # 16 contraction chunks
TQ = 512              # token-chunk
NCH = BT // TQ        # 8 token chunks == NCORES
NCORES = 8
NTOK = BT // NCORES   # 512 tokens per core for the output projection

f32 = mybir.dt.float32
f32r = mybir.dt.float32r
bf16 = mybir.dt.bfloat16
EXP = mybir.ActivationFunctionType.Exp
SCALE = float(HD) ** -0.5


def split_multi_waits(nc):
    """Walrus codegen allows only one sync-wait per engine instruction; move
    extras onto standalone same-engine EventSemaphore waits placed before."""
    for fn in nc.m.functions:
        for bb in fn.blocks:
            out = []
            for inst in bb.instructions:
                si = inst.sync_info
                if si is not None and si.on_wait and len(si.on_wait) > 1:
                    waits = list(si.on_wait)
                    for j, w in enumerate(waits[:-1]):
                        nop = mybir.InstEventSemaphore(
                            name=f"{inst.name}-ws{j}", ins=[], outs=[],
                            engine=inst.engine)
                        nop.sync_info = mybir.SyncInfo(on_wait=[w], on_update=[])
                        out.append(nop)
                    inst.sync_info = mybir.SyncInfo(
                        on_wait=[waits[-1]], on_update=list(si.on_update))
                out.append(inst)
            try:
                bb.instructions[:] = out
            except TypeError:
                bb.instructions.clear()
                bb.instructions.extend(out)


def build(n_rep=1):
    nc = bass.Bass(num_devices=NCORES)

    xt_d = nc.dram_tensor("xt", [C, BT], bf16, kind="ExternalInput")
    wq_d = nc.dram_tensor("wq", [C, CQ], bf16, kind="ExternalInput")
    wkv_d = nc.dram_tensor("wkv", [C, P], bf16, kind="ExternalInput")
    wpp_d = nc.dram_tensor("wpp", [C, C], bf16, kind="ExternalInput")
    bpb_d = nc.dram_tensor("bpb", [P, C], f32, kind="ExternalInput")
    mask_d = nc.dram_tensor("masks", [P, G * TQ], bf16, kind="ExternalInput")
    idn_d = nc.dram_tensor("ident", [P, P], f32, kind="ExternalInput")
    ones_d = nc.dram_tensor("ones", [1, HD], f32r, kind="ExternalInput")
    vpad_d = nc.dram_tensor("vpad", [P, (BT // P) * 2], bf16,
                            kind="ExternalInput")
    out_d = nc.dram_tensor("out", [NTOK, C], f32, kind="ExternalOutput")

    with tile.TileContext(nc) as tc:
      with tc.tile_pool(name="dram", bufs=1, space="DRAM") as dp:
        for rep in range(n_rep):
          with tc.tile_pool(name="act", bufs=1) as act:
            # phase-2/3 constants: tiles here, DMAs interleaved into phase 1
            # so the first xt/wq transfers aren't stuck behind them
            idn_sb = act.tile([P, P], f32)
            ones_sb = act.tile([1, HD], f32r)
            mask_sb = act.tile([P, G, TQ], bf16)
            bpb_sb = act.tile([P, C], f32)
            wp_sb = act.tile([P, KC, C], bf16)
            # long-lived activations; one qT tile per head so every matmul
            # operand sits at base partition 0
            qTh = [act.tile([HD, BT], bf16, name=f"qt{rep}_{h}")
                   for h in range(G)]
            kT = act.tile([HD, BT], bf16)
            va = act.tile([P, BT // P, HD + 2], bf16)  # v natural + ones col
            nc.sync.dma_start(
                va[:, :, HD:HD + 2],
                vpad_d.rearrange("p (k c) -> p k c", c=2))
            yt_sb = act.tile([P, KC, TQ], bf16)  # A2A result (proj lhsT)
            # A2A stages {h0,h1}, {h2}, {h3}: one contiguous dest-major DRAM
            # tile per stage so a stage's collective read doesn't falsely
            # overlap later heads' writes
            STAGES = ((0, 2), (2, 3), (3, 4))
            st_loc = [dp.tile([NCORES, hi - lo, HD, TQ], bf16,
                              name=f"stl{rep}_{si}")
                      for si, (lo, hi) in enumerate(STAGES)]
            yt_a2a = dp.tile([C, TQ], bf16)

            # ---- Phase 1: q/k/v projections (contract C on partitions) ----
            with tc.tile_pool(name="xp", bufs=4) as xp, \
                 tc.tile_pool(name="w1", bufs=1) as w1, \
                 tc.tile_pool(name="pps", bufs=2, space="PSUM") as pps, \
                 tc.tile_pool(name="tps", bufs=2, space="PSUM") as tps:
                # DMA order is the phase-1 critical path: the first two wq/wkv
                # chunks and the first xt tile go first, then the weight
                # remainders interleave behind tb0's xt stream (each DMA pays
                # ~625ns of serialized HWDGE, so order = PE start time)
                wq_sb = w1.tile([P, KC, CQ], bf16)
                wkv_sb = w1.tile([P, KC, P], bf16)

                def _wslice(dst, src, c0, c1):
                    nc.sync.dma_start(
                        dst[:, c0:c1, :],
                        src[c0 * P:c1 * P, :].rearrange("(o p) n -> p o n", p=P))

                _wslice(wq_sb, wq_d, 0, 2)
                vs_q = []
                warm = w1.tile([1, 2], f32)
                for tb in range(BT // TQ):
                    if tb == 2:
                        # prewarm the Exp activation table while ACT is idle
                        nc.scalar.activation(warm[:], idn_sb[0:1, 0:2], EXP)
                    q0_ps = pps.tile([P, TQ], f32, tag="q0")
                    q1_ps = pps.tile([P, TQ], f32, tag="q1")
                    kv_ps = pps.tile([P, TQ], f32, tag="kv")
                    for cg in range(KC // 4):
                        xt_t = xp.tile([P, 4, TQ], bf16, tag="xt")
                        nc.sync.dma_start(
                            xt_t[:],
                            xt_d[cg * 4 * P:(cg + 1) * 4 * P,
                                 tb * TQ:(tb + 1) * TQ].rearrange(
                                "(o p) n -> p o n", p=P))
                        if tb == 0:
                            if cg == 0:
                                # cg0 consumes c0-3: these slices must be
                                # issued before its matmuls are emitted
                                _wslice(wkv_sb, wkv_d, 0, 6)
                                _wslice(wq_sb, wq_d, 2, 6)
                            elif cg == 1:
                                _wslice(wq_sb, wq_d, 6, KC)
                                _wslice(wkv_sb, wkv_d, 6, KC)
                            elif cg == 2:
                                # consumed by tb0's (deferred) transposes
                                nc.sync.dma_start(idn_sb[:], idn_d[:, :])
                                nc.sync.dma_start(ones_sb[:], ones_d[:, :])
                        elif cg == 3:
                            # phase-2/3 constants ride behind each tb's xt
                            # stream so they never delay the PE feed
                            if tb == 1:
                                nc.sync.dma_start(
                                    mask_sb[:],
                                    mask_d.rearrange("p (g t) -> p g t", g=G))
                                nc.sync.dma_start(bpb_sb[:], bpb_d[:, :])
                            else:
                                for w2 in range(2):
                                    cw = (tb - 2) * 2 + w2
                                    nc.sync.dma_start(
                                        wp_sb[:, cw, :],
                                        wpp_d[cw * P:(cw + 1) * P, :])
                        for cc in range(4):
                            c = cg * 4 + cc
                            nc.tensor.matmul(q0_ps[:], wq_sb[:, c, 0:P],
                                             xt_t[:, cc, :],
                                             start=(c == 0), stop=(c == KC - 1))
                            nc.tensor.matmul(q1_ps[:], wq_sb[:, c, P:CQ],
                                             xt_t[:, cc, :],
                                             start=(c == 0), stop=(c == KC - 1))
                            nc.tensor.matmul(kv_ps[:], wkv_sb[:, c, :],
                                             xt_t[:, cc, :],
                                             start=(c == 0), stop=(c == KC - 1))
                    sl = slice(tb * TQ, (tb + 1) * TQ)
                    nc.vector.tensor_copy(qTh[0][:, sl], q0_ps[0:HD, :])
                    nc.vector.tensor_copy(qTh[1][:, sl], q0_ps[HD:P, :])
                    nc.vector.tensor_copy(qTh[2][:, sl], q1_ps[0:HD, :])
                    nc.vector.tensor_copy(qTh[3][:, sl], q1_ps[HD:P, :])
                    nc.vector.tensor_copy(kT[:, sl], kv_ps[0:HD, :])
                    vs_t = xp.tile([HD, TQ], f32, tag="vs")
                    nc.vector.tensor_copy(vs_t[:], kv_ps[HD:P, :])
                    # V transposes deferred one tb so they overlap the next
                    # tb's matmuls instead of stalling PE on the vs copy
                    vs_q.append((tb, vs_t))
                    if len(vs_q) == 2:
                        dtb, dvs = vs_q.pop(0)
                        for k4 in range(TQ // P):
                            kb = dtb * (TQ // P) + k4
                            vt_ps = tps.tile([P, HD], f32, tag="vt")
                            nc.tensor.transpose(vt_ps[:],
                                                dvs[:, k4 * P:(k4 + 1) * P],
                                                idn_sb[0:HD, 0:HD])
                            nc.vector.tensor_copy(va[:, kb, 0:HD], vt_ps[:])
                dtb, dvs = vs_q.pop(0)
                for k4 in range(TQ // P):
                    kb = dtb * (TQ // P) + k4
                    vt_ps = tps.tile([P, HD], f32, tag="vt")
                    nc.tensor.transpose(vt_ps[:], dvs[:, k4 * P:(k4 + 1) * P],
                                        idn_sb[0:HD, 0:HD])
                    nc.vector.tensor_copy(va[:, kb, 0:HD], vt_ps[:])
                for cw in range(12, KC):
                    nc.sync.dma_start(
                        wp_sb[:, cw, :], wpp_d[cw * P:(cw + 1) * P, :])

            # ---- Phase 2: causal attention (scores-transposed, trimmed),
            # head-staged normalize + AllToAll overlapped with next head ----
            with tc.tile_pool(name="aps", bufs=2, space="PSUM") as aps, \
                 tc.tile_pool(name="yps", bufs=2, space="PSUM") as yps, \
                 tc.tile_pool(name="bps", bufs=2, space="PSUM") as bps, \
                 tc.tile_pool(name="ep", bufs=6) as ep, \
                 tc.tile_pool(name="np_", bufs=4) as npo:
                def emit_y(pend):
                    """mask + V-accumulate for a finished score pair."""
                    y_ps, ex, b, qc, kbp, nkb = pend
                    for h in range(2):
                        kb = 2 * kbp + h
                        j = kb - 4 * qc
                        off = P * max(0, j)
                        if j >= 0:
                            nc.vector.tensor_mul(
                                ex[:, h, off:off + P],
                                ex[:, h, off:off + P],
                                mask_sb[:, j, off:off + P])
                        nc.tensor.matmul(
                            y_ps[:, off:],
                            va[:, b * (T // P) + kb, :],
                            ex[:, h, off:],
                            start=(kb == 0), stop=(kb == nkb - 1),
                            skip_group_check=True)

                def emit_norm(pend):
                    """r = 1/l, ones-matmul broadcast, scale, ship to stage."""
                    y_ps, qh, ci = pend
                    rrow = npo.tile([1, TQ], f32r, tag="rr", bufs=2)
                    with nc.allow_low_precision(
                            reason="1/l as f32r matmul operand (~5e-6)"):
                        nc.vector.reciprocal(rrow[:], y_ps[HD:HD + 1, :])
                    ystage = npo.tile([HD, TQ], bf16, tag="ys", bufs=2)
                    nc.vector.tensor_copy(ystage[:], y_ps[0:HD, :])
                    bc_ps = bps.tile([HD, TQ], f32, tag="bc")
                    nc.tensor.matmul(bc_ps[:], ones_sb[:], rrow[:],
                                     start=True, stop=True)
                    yn = npo.tile([HD, TQ], bf16, tag="yn")
                    nc.vector.tensor_mul(yn[:], ystage[:], bc_ps[:])
                    si = 0 if qh < 2 else qh - 1
                    nc.sync.dma_start(
                        st_loc[si][ci, qh - STAGES[si][0], :, :], yn[:])

                # software-pipelined: score matmuls run one pair ahead of the
                # V-accumulates (and the previous chunk's normalize slots in
                # after the first pair) so the in-order PE queue never waits
                # on an exp that hasn't finished
                pend_y = None
                pend_norm = None
                for qh in range(G):
                    for ci in range(NCH):
                        b, qc = ci // (T // TQ), ci % (T // TQ)
                        nkb = 4 * qc + 4
                        y_ps = yps.tile([HD + 2, TQ], f32, tag="y")
                        qap = qTh[qh][:, b * T + qc * TQ: b * T + (qc + 1) * TQ]
                        for kbp in range(nkb // 2):
                            j0 = 2 * kbp - 4 * qc
                            offp = P * max(0, j0)
                            s_ps = aps.tile([P, 2, TQ], f32, tag="s")
                            for h in range(2):
                                kb = 2 * kbp + h
                                off = P * max(0, kb - 4 * qc)
                                nc.tensor.matmul(
                                    s_ps[:, h, off:],
                                    kT[:, b * T + kb * P: b * T + (kb + 1) * P],
                                    qap[:, off:], start=True, stop=True)
                            ex = ep.tile([P, 2, TQ], bf16, tag="ex")
                            nc.scalar.activation(ex[:, :, offp:],
                                                 s_ps[:, :, offp:], EXP,
                                                 scale=SCALE)
                            if pend_y is not None:
                                emit_y(pend_y)
                            pend_y = (y_ps, ex, b, qc, kbp, nkb)
                            if pend_norm is not None:
                                emit_norm(pend_norm)
                                pend_norm = None
                        pend_norm = (y_ps, qh, ci)
                    # flush before the head's A2A: its stage needs all chunks
                    si = 0 if qh < 2 else qh - 1
                    lo, hi = STAGES[si]
                    if qh == hi - 1:  # stage complete: ship it
                        if pend_y is not None:
                            emit_y(pend_y)
                            pend_y = None
                        if pend_norm is not None:
                            emit_norm(pend_norm)
                            pend_norm = None
                        r0, r1 = lo * NCORES * HD, hi * NCORES * HD
                        nc.gpsimd.collective_compute(
                            "AllToAll", mybir.AluOpType.bypass,
                            replica_groups=[list(range(NCORES))],
                            ins=[st_loc[si][:].opt()],
                            outs=[yt_a2a[r0:r1, :].opt()])
                # land stage channels for the projection, on the gpsimd queue
                # (in-order behind the last A2A, so no engine stalls on the
                # completion wait)
                for (r0, r1) in ((0, 1024), (1024, 1536), (1536, 2048)):
                    nc.gpsimd.dma_start(
                        yt_sb[:, r0 // P:r1 // P, :],
                        yt_a2a[r0:r1, :].rearrange("(c p) t -> p c t", p=P))

            # ---- Phase 3: token-sharded output projection + bias ----
            # Column-groups of 512 out-channels x all 4 token blocks (4 PSUM
            # banks each, 2 groups in flight). All groups pre-accumulate the
            # A2A stage-0..2 channels (c0-11) while the last A2A is in
            # flight; groups 0/1 park their partials in SBUF and preload
            # them back into PSUM for the c12-15 finish.
            NG = C // TQ
            with tc.tile_pool(name="fps", bufs=2, space="PSUM") as fps, \
                 tc.tile_pool(name="fp", bufs=2) as fp:
                keep = {}
                dumps = {}
                CPRE = 12  # chunks from A2A stages 0-2

                def proj_chunks(ops, g, c0, c1, start):
                    osl = slice(g * TQ, (g + 1) * TQ)
                    for c in range(c0, c1):
                        for i in range(4):
                            nc.tensor.matmul(
                                ops[i][:], yt_sb[:, c, i * P:(i + 1) * P],
                                wp_sb[:, c, osl],
                                start=start and (c == c0), stop=(c == c1 - 1),
                                skip_group_check=True)

                for g in range(NG):
                    ops = [fps.tile([P, TQ], f32, tag=f"t{i}",
                                    name=f"o{rep}_{g}_{i}") for i in range(4)]
                    proj_chunks(ops, g, 0, CPRE, start=True)
                    if g < 2:
                        dsb = fp.tile([P, 4, TQ], f32, tag=f"d{g}", bufs=1,
                                      name=f"d{rep}_{g}")
                        for i in range(4):
                            nc.vector.tensor_copy(dsb[:, i, :], ops[i][:])
                        dumps[g] = dsb
                    else:
                        keep[g] = ops
                for g in (2, 3, 0, 1):
                    osl = slice(g * TQ, (g + 1) * TQ)
                    if g < 2:
                        ops = [fps.tile([P, TQ], f32, tag=f"t{i}",
                                        name=f"p{rep}_{g}_{i}")
                               for i in range(4)]
                        for i in range(4):
                            nc.vector.tensor_copy(ops[i][:], dumps[g][:, i, :])
                    else:
                        ops = keep[g]
                    proj_chunks(ops, g, CPRE, KC, start=False)
                    for i in range(4):
                        o_sb = fp.tile([P, TQ], f32, tag="ob", bufs=4)
                        nc.vector.tensor_add(o_sb[:], ops[i][:],
                                             bpb_sb[:, osl])
                        nc.sync.dma_start(
                            out_d[i * P:(i + 1) * P, osl], o_sb[:])

    split_multi_waits(nc)
    return nc


_NC_CACHE = {}


def _get_nc(n_rep=1):
    if n_rep not in _NC_CACHE:
        _NC_CACHE[n_rep] = build(n_rep)
    return _NC_CACHE[n_rep]


def make_in_maps(x, wq, wk, wv, wp, bp):
    x = np.asarray(x, dtype=np.float32)
    xt = np.ascontiguousarray(x.reshape(BT, C).T).astype(ml_dtypes.bfloat16)
    masks = np.zeros((P, G * TQ), dtype=np.float32)
    for j in range(G):
        kk = np.arange(P)[:, None]
        qq = np.arange(TQ)[None, :]
        masks[:, j * TQ:(j + 1) * TQ] = (j * P + kk <= qq).astype(np.float32)
    masks = masks.astype(ml_dtypes.bfloat16)
    ident = np.eye(P, dtype=np.float32)
    ones = np.ones((1, HD), dtype=np.float32)
    vpad = np.tile(np.array([1.0, 0.0], dtype=np.float32),
                   (P, BT // P)).astype(ml_dtypes.bfloat16)
    # wp row permutation matching the A2A arrival order:
    # stage {h0,h1}: (src core j, h in {0,1}, d); stages {h2}, {h3}: (j, d)
    perm = ([(j * G + h) * HD + d
             for j in range(NCORES) for h in range(2) for d in range(HD)] +
            [(j * G + h) * HD + d
             for h in (2, 3) for j in range(NCORES) for d in range(HD)])
    perm = np.array(perm)
    wpp = np.ascontiguousarray(
        np.asarray(wp, np.float32)[perm, :]).astype(ml_dtypes.bfloat16)
    bpb = np.tile(np.asarray(bp, np.float32)[None, :], (P, 1))
    in_maps = []
    for i in range(NCORES):
        cs = slice(i * CQ, (i + 1) * CQ)
        ks = slice(i * HD, (i + 1) * HD)
        wkv = np.concatenate(
            [np.asarray(wk)[:, ks], np.asarray(wv)[:, ks]], axis=1)
        in_maps.append({
            "xt": xt,
            "wq": np.ascontiguousarray(
                np.asarray(wq, np.float32)[:, cs]).astype(ml_dtypes.bfloat16),
            "wkv": np.ascontiguousarray(
                wkv.astype(np.float32)).astype(ml_dtypes.bfloat16),
            "wpp": wpp,
            "bpb": bpb,
            "masks": masks,
            "ident": ident,
            "ones": ones,
            "vpad": vpad,
        })
    return in_maps


def kernel(x, wq, wk, wv, wp, bp, _trace=False):
    from concourse.bass_utils import run_bass_kernel_spmd
    nc = _get_nc()
    in_maps = make_in_maps(x, wq, wk, wv, wp, bp)
    res = run_bass_kernel_spmd(nc, in_maps, list(range(NCORES)), trace=_trace)
    out = np.concatenate([res.results[i]["out"] for i in range(NCORES)], axis=0)
    out = out.reshape(B, T, C).astype(np.float32)
    if _trace:
        return out, res
    return out
